# revision 10
# baseline (speedup 1.0000x reference)
"""Fused Linear + LayerNorm + residual-multiply kernel for 8 Trainium2 cores.

Computes, for full inputs x[B,1024], y[B,1024], weight[1024,1024], bias, gamma, beta:
    z  = x @ weight.T + bias
    ln = (z - mean(z)) * rsqrt(var(z) + eps) * gamma + beta     (over last dim)
    out = (ln + y) * y

Data-parallel over the batch dim: each of the 8 NeuronCores processes B/8 rows;
weight/bias/gamma/beta are replicated. No cross-core communication.

Fast path (gamma==1, beta==0), built around:

1. fp8 DoubleRow matmuls. x and W.T are quantized to fp8-e4m3 on the host
   (W.T pre-scaled by 32 so LayerNorm's scale-invariance absorbs it; only eps
   needs compensating: eps' = 32^2 * eps). DoubleRow packs two contraction
   rows per PE cell -> K=256 per matmul, halving matmul instruction count.
   The per-row bias is added by K=1 ones x (32*bias) fp16 matmuls issued with
   start=True before x arrives.

2. A five-engine consumer, software-pipelined by one tile (engines are strict
   FIFO, so each cross-engine dependency gets a full tile-period of slack):
     ScalarE : t0 = fp16(z') with accum_out -> sum(z');
               second pass applies the affine (z'-mean)*rstd from PSUM.
     GpSimd  : sumsq via (t0*0.25)*t0 with accum_out (otherwise idle engine).
     VectorE : [P,1] stat smalls + the two fp16 tensor_tensor ops
               (t+y) and (.)*y, which hit the DVE 2x mode.
   Per-partition-scalar application on [P,D] data stays on ScalarE
   (DVE scalar_tensor_tensor measures 1x on hardware for full tiles).

3. Batched DMA: one descriptor set per 512-row chunk per tensor (contiguous
   multi-KB per partition, packed on the host). Inputs ride the sync ring
   (first y chunk on the scalar ring to shorten the head), outputs the scalar
   ring; the last chunk stores per-tile to shorten the drain tail. Output is
   fp16, upcast on the host.
"""

import numpy as np
import ml_dtypes
from contextlib import ExitStack

import concourse.bass as bass
import concourse.mybir as mybir
import concourse.tile as tile
from concourse import bacc, bass_utils


P = 128
D = 1024
KT = D // P          # 8 k-tiles over the contraction dim
KK = KT // 2         # 4 DoubleRow pairs
OB = 512             # o-block width (one PSUM bank of fp32)
ST = 512             # rows per super-chunk
TPC = ST // P        # 4 tiles per chunk
N_CORES = 8
EPS = 1e-5
W_SCALE = 32.0       # W.T pre-scale so fp8 entries stay in normal range
EPS_DEV = EPS * W_SCALE * W_SCALE   # eps seen by the scaled z' = 32z
SQ_SCALE = 0.25      # sq pass computes (t0*0.25)*t0 -> sumsq' = 4 * accum

F32 = mybir.dt.float32
F16 = mybir.dt.float16
F8 = mybir.dt.float8e4

AF = mybir.ActivationFunctionType
OP = mybir.AluOpType
DR = mybir.MatmulPerfMode.DoubleRow

_BUILD_CACHE = {}


def _build(b_core: int, trivial_affine: bool):
    key = (b_core, trivial_affine)
    if key in _BUILD_CACHE:
        return _BUILD_CACHE[key]

    nst = b_core // ST
    nb = b_core // P
    nc = bacc.Bacc("TRN2", debug=False, num_devices=N_CORES)

    # x.T packed fp8: xt[st, p, k*ST + b] = x.T[k*P + p, st*ST + b]
    xt = nc.dram_tensor("xt", [nst, P, KT * ST], F8, kind="ExternalInput").ap()
    # y packed fp16: yh[st, p, t*D + o] = y[st*ST + t*P + p, o]
    yh = nc.dram_tensor("yh", [nst, P, TPC * D], F16, kind="ExternalInput").ap()
    # W.T packed fp8 (pre-scaled by 32): wth[p, k*D + o] = 32 * W.T[k*P + p, o]
    wth = nc.dram_tensor("wth", [P, KT * D], F8, kind="ExternalInput").ap()
    biash = nc.dram_tensor("biash", [D], F16, kind="ExternalInput").ap()  # 32*bias
    if not trivial_affine:
        gamma = nc.dram_tensor("gamma", [D], F32, kind="ExternalInput").ap()
        beta = nc.dram_tensor("beta", [D], F32, kind="ExternalInput").ap()
    # out packed fp16: outh[st, p, t*D + o] = out[st*ST + t*P + p, o]
    outh = nc.dram_tensor("outh", [nst, P, TPC * D], F16, kind="ExternalOutput").ap()

    with tile.TileContext(nc) as tc, ExitStack() as ctx:
        const = ctx.enter_context(tc.tile_pool(name="const", bufs=1))
        xtp = ctx.enter_context(tc.tile_pool(name="xtp", bufs=2))
        ypool = ctx.enter_context(tc.tile_pool(name="yp", bufs=2))
        tpool = ctx.enter_context(tc.tile_pool(name="tp", bufs=3))
        jpool = ctx.enter_context(tc.tile_pool(name="jp", bufs=3))
        npool = ctx.enter_context(tc.tile_pool(name="np", bufs=3))
        opool = ctx.enter_context(tc.tile_pool(name="op", bufs=2))
        stat = ctx.enter_context(tc.tile_pool(name="stat", bufs=10))
        psz = ctx.enter_context(tc.tile_pool(name="psz", bufs=4, space="PSUM"))

        # --- constants ---
        bias_sb = const.tile([1, D], F16)
        nc.scalar.dma_start(out=bias_sb[:], in_=biash.unsqueeze(0))
        wt_sb = const.tile([P, KT, D], F8)  # [i_local, k, o]
        wt_src = wth.rearrange("p (k o) -> p k o", k=KT)
        # halves so kk 0/1 matmuls only wait on the first half
        nc.sync.dma_start(out=wt_sb[:, 0 : KT // 2, :], in_=wt_src[:, 0 : KT // 2, :])
        nc.sync.dma_start(out=wt_sb[:, KT // 2 :, :], in_=wt_src[:, KT // 2 :, :])
        ones_f32 = const.tile([1, P], F32)
        nc.vector.memset(ones_f32[:], 1.0)
        ones_sb = const.tile([1, P], F16)
        nc.scalar.activation(ones_sb[:], ones_f32[:], AF.Copy)
        eps_sb = const.tile([P, 1], F32)
        nc.vector.memset(eps_sb[:], EPS_DEV)
        if not trivial_affine:
            gamma_f32 = const.tile([P, D], F32)
            nc.sync.dma_start(out=gamma_f32[:], in_=gamma.unsqueeze(0).to_broadcast([P, D]))
            gamma_sb = const.tile([P, D], F16)
            nc.scalar.activation(gamma_sb[:], gamma_f32[:], AF.Copy)
            beta_f32 = const.tile([P, D], F32)
            nc.sync.dma_start(out=beta_f32[:], in_=beta.unsqueeze(0).to_broadcast([P, D]))
            beta_sb = const.tile([P, D], F16)
            nc.scalar.activation(beta_sb[:], beta_f32[:], AF.Copy)

        # --- PE warmup: keep the HAM activity monitor busy during input
        # staging so the real matmuls start at 2.4 GHz instead of 1.2 GHz.
        warm_mov = const.tile([1, OB], F32)
        nc.vector.memset(warm_mov[:], 0.0)
        warm_ps = psz.tile([P, D], F32, tag="z_ps")
        for w in range(2):
            nc.tensor.matmul(
                warm_ps[:, 0:OB], ones_f32[:], warm_mov[:], start=True, stop=True
            )

        # Per-tile state for the 1-tile software pipeline.
        tiles = [dict() for _ in range(nb)]
        xt_sbs = [None] * nst
        y_sbs = [None] * nst
        o_sbs = [None] * nst

        def produce(bt):
            st, t = divmod(bt, TPC)
            tl = tiles[bt]
            if t == 0:
                xt_sbs[st] = xtp.tile([P, KT, ST], F8, name=f"xt_sb{st}")
                nc.sync.dma_start(
                    out=xt_sbs[st][:], in_=xt[st].rearrange("p (k b) -> p k b", k=KT)
                )
                # flat 2D tiles: slices keep clean 2D APs so DVE 2x mode fires
                y_sbs[st] = ypool.tile([P, TPC * D], F16, name=f"y_sb{st}")
                ring = nc.scalar if st == 0 else nc.sync
                ring.dma_start(out=y_sbs[st][:], in_=yh[st])
                o_sbs[st] = opool.tile([P, TPC * D], F16, name=f"o_sb{st}")
            tl["y"] = y_sbs[st][:, bass.ts(t, D)]
            tl["o"] = o_sbs[st][:, bass.ts(t, D)]
            tl["st"], tl["t"] = st, t

            z_ps = psz.tile([P, D], F32)
            tl["z"] = z_ps
            # bias first (start=True) so it can issue before x arrives
            for half in range(2):
                nc.tensor.matmul(
                    z_ps[:, bass.ts(half, OB)],
                    ones_sb[:],
                    bias_sb[:, bass.ts(half, OB)],
                    start=True,
                    stop=False,
                )
            for kk in range(KK):
                ksl = slice(2 * kk, 2 * kk + 2)
                lhsT = xt_sbs[st][:, ksl, bass.ts(t, P)]
                for half in range(2):
                    nc.tensor.matmul(
                        z_ps[:, bass.ts(half, OB)],
                        lhsT,
                        wt_sb[:, ksl, bass.ts(half, OB)],
                        start=False,
                        stop=(kk == KK - 1),
                        perf_mode=DR,
                    )

            if trivial_affine:
                # stats straight off PSUM (exact fp32 z')
                stt = stat.tile([P, 2, 6], F32)
                for half in range(2):
                    nc.vector.bn_stats(
                        out=stt[:, half, :], in_=z_ps[:, bass.ts(half, OB)]
                    )
                mv = stat.tile([P, 2], F32)
                nc.vector.bn_aggr(out=mv[:], in_=stt[:])
                tl["mv"] = mv

        def mid(bt):
            # stage B: stat smalls + affine normalize + residual add
            tl = tiles[bt]
            z_ps, y_t = tl["z"], tl["y"]
            mv = tl["mv"]
            negmean = stat.tile([P, 1], F32)
            nc.scalar.activation(negmean[:], mv[:, 0:1], AF.Copy, scale=-1.0)
            std = stat.tile([P, 1], F32)
            nc.scalar.activation(
                std[:], mv[:, 1:2], AF.Sqrt, scale=1.0, bias=eps_sb[:]
            )
            # one GpSimd op: nmr = -mean'/std'; std is OVERWRITTEN with rstd'
            nmr = stat.tile([P, 1], F32)
            nc.gpsimd.normalize_recip(nmr[:], negmean[:], std[:])
            # affine from PSUM: tnorm = (z' - mean')*rstd'   (ScalarE)
            tnorm = npool.tile([P, D], F16)
            nc.scalar.activation(
                tnorm[:], z_ps[:], AF.Identity, bias=nmr[:], scale=std[:]
            )
            # residual add on GpSimd (otherwise idle)
            u = jpool.tile([P, D], F16)
            nc.gpsimd.tensor_add(out=u[:], in0=tnorm[:], in1=y_t)
            tl["u"] = u

        def late(bt):
            # stage C: final multiply + store
            tl = tiles[bt]
            st, t = tl["st"], tl["t"]
            nc.vector.tensor_mul(out=tl["o"], in0=tl["u"][:], in1=tl["y"])
            if st == nst - 1:
                # last chunk: per-tile stores to shorten the drain tail
                nc.scalar.dma_start(
                    out=outh[st, :, bass.ts(t, D)], in_=o_sbs[st][:, bass.ts(t, D)]
                )
            elif t == TPC - 1:
                nc.scalar.dma_start(out=outh[st], in_=o_sbs[st][:])

        def consume(bt):
            tl = tiles[bt]
            st, t = tl["st"], tl["t"]
            z_ps, y_t, o_t = tl["z"], tl["y"], tl["o"]
            if True:
                stt = stat.tile([P, 2, 6], F32)
                for half in range(2):
                    nc.vector.bn_stats(
                        out=stt[:, half, :], in_=z_ps[:, bass.ts(half, OB)]
                    )
                mv = stat.tile([P, 2], F32)
                nc.vector.bn_aggr(out=mv[:], in_=stt[:])
                std = stat.tile([P, 1], F32)
                nc.scalar.activation(
                    std[:], mv[:, 1:2], AF.Sqrt, bias=eps_sb[:], scale=1.0
                )
                rstd = stat.tile([P, 1], F32)
                nc.vector.reciprocal(rstd[:], std[:])
                nmr = stat.tile([P, 1], F32)
                nc.vector.scalar_tensor_tensor(
                    out=nmr[:], in0=mv[:, 0:1], scalar=-1.0, in1=rstd[:],
                    op0=OP.mult, op1=OP.mult,
                )
                tnorm = npool.tile([P, D], F16)
                nc.scalar.activation(
                    tnorm[:], z_ps[:], AF.Identity, bias=nmr[:], scale=rstd[:]
                )
                nc.vector.tensor_mul(out=tnorm[:], in0=tnorm[:], in1=gamma_sb[:])
                nc.vector.tensor_add(out=tnorm[:], in0=tnorm[:], in1=beta_sb[:])
                u = npool.tile([P, D], F16)
                nc.vector.tensor_add(out=u[:], in0=tnorm[:], in1=y_t)
                nc.vector.tensor_mul(out=o_t, in0=u[:], in1=y_t)

            if st == nst - 1:
                # last chunk: per-tile stores to shorten the drain tail
                nc.scalar.dma_start(
                    out=outh[st, :, bass.ts(t, D)], in_=o_sbs[st][:, bass.ts(t, D)]
                )
            elif t == TPC - 1:
                nc.scalar.dma_start(out=outh[st], in_=o_sbs[st][:])

        if trivial_affine:
            for bt in range(nb):
                produce(bt)
                if bt >= 1:
                    mid(bt - 1)
                if bt >= 2:
                    late(bt - 2)
            mid(nb - 1)
            late(nb - 2)
            late(nb - 1)
        else:
            for bt in range(nb):
                produce(bt)
                if bt > 0:
                    consume(bt - 1)
            consume(nb - 1)

    nc.finalize()
    _BUILD_CACHE[key] = nc
    return nc


def _run(nc, in_maps, **kwargs):
    return bass_utils.run_bass_kernel_spmd(
        nc, in_maps, core_ids=list(range(N_CORES)), **kwargs
    )


def _prepare(x, y, weight, bias, gamma, beta):
    x = np.asarray(x, dtype=np.float32)
    y = np.ascontiguousarray(y, dtype=np.float32)
    weight = np.asarray(weight, dtype=np.float32)
    bias = np.asarray(bias, dtype=np.float32)
    gamma = np.asarray(gamma, dtype=np.float32)
    beta = np.asarray(beta, dtype=np.float32)

    B, IN = x.shape
    assert IN == D and weight.shape == (D, D) and y.shape == (B, D)
    assert B % (N_CORES * ST) == 0
    b_core = B // N_CORES
    nst = b_core // ST

    trivial = bool(np.all(gamma == 1.0)) and bool(np.all(beta == 0.0))
    nc = _build(b_core, trivial)

    fp8 = ml_dtypes.float8_e4m3fn
    # W.T packed fp8: wth[p, k*D + o] = 32 * W.T[k*P+p, o] = 32 * weight[o, k*P+p]
    wt = (weight.T * W_SCALE).astype(fp8)            # [i, o] = [k*P+p, o]
    wth_prep = np.ascontiguousarray(
        wt.reshape(KT, P, D).transpose(1, 0, 2)
    ).reshape(P, KT * D)
    biash = (bias * W_SCALE).astype(np.float16)
    in_maps = []
    for cid in range(N_CORES):
        xs = x[cid * b_core : (cid + 1) * b_core].astype(fp8)
        # x.T packed fp8: xt[st, p, k*ST + b] = x.T[k*P+p, st*ST + b]
        xt_prep = np.ascontiguousarray(
            xs.T.reshape(KT, P, nst, ST).transpose(2, 1, 0, 3)
        ).reshape(nst, P, KT * ST)
        ys = y[cid * b_core : (cid + 1) * b_core].astype(np.float16)
        # y packed: yh[st, p, t*D + o] = y[st*ST + t*P + p, o]
        yh_prep = np.ascontiguousarray(
            ys.reshape(nst, TPC, P, D).transpose(0, 2, 1, 3)
        ).reshape(nst, P, TPC * D)
        m = {
            "xt": xt_prep,
            "yh": yh_prep,
            "wth": wth_prep,
            "biash": biash,
        }
        if not trivial:
            m["gamma"] = gamma
            m["beta"] = beta
        in_maps.append(m)
    return nc, in_maps


def kernel(x, y, weight, bias, gamma, beta):
    nc, in_maps = _prepare(x, y, weight, bias, gamma, beta)
    res = _run(nc, in_maps)
    B = np.asarray(x).shape[0]
    b_core = B // N_CORES
    nst = b_core // ST
    outs = []
    for r in res.results:
        # unpack outh[st, p, t*D + o] -> out[st*ST + t*P + p, o]
        oh = np.asarray(r["outh"]).astype(np.float32)
        oh = oh.reshape(nst, P, TPC, D).transpose(0, 2, 1, 3).reshape(b_core, D)
        outs.append(oh)
    return np.concatenate(outs, axis=0)
